# revision 1
# baseline (speedup 1.0000x reference)
"""Trainium2 (8 NeuronCores) kernel for coverage attention — v5.

feat.T layout, host-folded coverage, host-transposed x, bf16 matmuls.
All instruction types are hardware-proven (v1 vocabulary): matmul with
PSUM accumulate, activation with per-partition bias reading PSUM,
tensor_copy, tensor_reduce, DMA.

Per-core pipeline, for each (batch b, 512-seq group g):
  - DMA x~T slab [128h, 4k, 512s] bf16  (x~ = x + cov (x) u folded on
    host via u = Wc[0] @ Wh^{-1}, pre-transposed to [H, S] per batch)
  - for m in 4: PSUM[128 h_m, 512 s] = sum_k Wh[k, ms].T @ x~T_k
    (4 bf16 matmuls, 512 cyc each); tanh via ACT with bias
    A.T[ms, b] (per-partition — free); e chunk accumulated on PE:
    ps_e[1, 512] += vw[ms].T @ f_m  (bf16)
  - e row -> e_sb[b] via partition-0 scratch + SB->SB DMA (v1 pattern)
Epilogue: v1's row softmax on [bpc, S] (max-reduce, Exp w/ accum, recip,
scale).  sum_coverage = coverage + a_t on host.

Engine budget (cost model): PE 137us (20x512cyc/group), ACT 101us,
DVE ~30us, DMA ~55us.
"""

import os
import sys

for _p in ("/opt/trn_rl_repo", os.path.expanduser("~/.axon_site/_ro/trn_rl_repo")):
    if os.path.isdir(_p) and _p not in sys.path:
        sys.path.insert(0, _p)

import numpy as np

import concourse.bass as bass
from concourse import bacc
import concourse.tile as tile
from concourse import mybir

B, S, H = 64, 2048, 512
N_CORES = 8
BPC = B // N_CORES

FP = mybir.dt.float32
BF = mybir.dt.bfloat16

SLAB = 512
JT = SLAB // 128
NG = S // SLAB
HC = H // 128


def build_program(bpc=BPC, s=S):
    nc = bacc.Bacc(None)

    xt = nc.declare_dram_parameter("xt", [bpc * H, s], BF, isOutput=False)
    wh = nc.declare_dram_parameter("wh", [H, H], BF, isOutput=False)
    at = nc.declare_dram_parameter("at", [H, bpc], FP, isOutput=False)
    vwt = nc.declare_dram_parameter("vwt", [128, HC], BF, isOutput=False)
    out_a = nc.declare_dram_parameter("out_a", [bpc, s], FP, isOutput=True)

    from contextlib import ExitStack
    with tile.TileContext(nc) as tc, ExitStack() as ctx:
        const = ctx.enter_context(tc.tile_pool(name="const", bufs=1))
        xpool = ctx.enter_context(tc.tile_pool(name="xpool", bufs=3))
        fpool = ctx.enter_context(tc.tile_pool(name="fpool", bufs=9))
        egpool = ctx.enter_context(tc.tile_pool(name="egpool", bufs=3))
        psf_pool = ctx.enter_context(tc.tile_pool(name="ps_f", bufs=5, space="PSUM"))
        pse_pool = ctx.enter_context(tc.tile_pool(name="ps_e", bufs=3, space="PSUM"))

        # ---------------- preamble ----------------
        wh_sb = []
        for k in range(HC):
            t = const.tile([128, H], BF, tag=f"wh{k}", name=f"wh_sb{k}")
            nc.sync.dma_start(out=t, in_=wh[k * 128:(k + 1) * 128, :])
            wh_sb.append(t)
        at_sb = []
        for m in range(HC):
            t = const.tile([128, bpc], FP, tag=f"at{m}", name=f"at{m}")
            nc.sync.dma_start(out=t, in_=at[m * 128:(m + 1) * 128, :])
            at_sb.append(t)
        vwt_sb = const.tile([128, HC], BF, tag="vwt")
        nc.sync.dma_start(out=vwt_sb, in_=vwt[:, :])

        e_sb = const.tile([bpc, s], FP, tag="e_sb")

        # deferred e-dot matmuls: flushed one group late so their f
        # inputs are ready when PE reaches them (no head-of-line stall)
        pend = []

        def flush(item):
            fs, bb, gg = item
            ps_e = pse_pool.tile([1, SLAB], FP, tag="ps_e")
            for m in range(HC):
                nc.tensor.matmul(
                    ps_e,
                    vwt_sb[:, m:m + 1],
                    fs[m][:, :],
                    start=(m == 0),
                    stop=(m == HC - 1),
                )
            e_g = egpool.tile([1, SLAB], FP, tag="e_g")
            nc.vector.tensor_copy(e_g, ps_e)
            nc.sync.dma_start(
                out=e_sb[bb:bb + 1, gg * SLAB:(gg + 1) * SLAB], in_=e_g)

        # ---------------- main loop ----------------
        for b in range(bpc):
            for g in range(NG):
                src = xt[b * H:(b + 1) * H, g * SLAB:(g + 1) * SLAB]
                src = src.rearrange("(k p) s -> p k s", p=128)
                xs = xpool.tile([128, HC, SLAB], BF, tag="xs")
                nc.sync.dma_start(out=xs, in_=src)

                fs = []
                for m in range(HC):
                    ms = slice(m * 128, (m + 1) * 128)
                    ps = psf_pool.tile([128, SLAB], FP, tag="ps_f")
                    for k in range(HC):
                        nc.tensor.matmul(
                            ps,
                            wh_sb[k][:, ms],
                            xs[:, k, :],
                            start=(k == 0),
                            stop=(k == HC - 1),
                        )
                    f_m = fpool.tile([128, SLAB], BF, tag="f_m")
                    nc.scalar.activation(
                        out=f_m, in_=ps,
                        func=mybir.ActivationFunctionType.Tanh,
                        bias=at_sb[m][:, b:b + 1],
                    )
                    fs.append(f_m)
                pend.append((fs, b, g))
                if len(pend) > 1:
                    flush(pend.pop(0))
        while pend:
            flush(pend.pop(0))

        # ---------------- softmax + output (v1 pattern) ----------------
        smx = const.tile([bpc, 1], FP, tag="smx")
        nc.vector.tensor_reduce(
            out=smx, in_=e_sb, axis=mybir.AxisListType.X,
            op=mybir.AluOpType.max, negate=True,
        )
        p_sb = const.tile([bpc, s], FP, tag="p_sb")
        esum = const.tile([bpc, 1], FP, tag="esum")
        nc.scalar.activation(
            out=p_sb, in_=e_sb, func=mybir.ActivationFunctionType.Exp,
            bias=smx, accum_out=esum,
        )
        rsum = const.tile([bpc, 1], FP, tag="rsum")
        nc.vector.reciprocal(rsum, esum)
        a_out = const.tile([bpc, s], FP, tag="a_out")
        nc.vector.tensor_scalar_mul(a_out, p_sb, rsum)
        nc.sync.dma_start(out=out_a[:, :], in_=a_out)

    return nc


_PROG_CACHE = {}


def _get_program(key=(BPC, S)):
    if key not in _PROG_CACHE:
        nc = build_program(*key)
        nc.finalize()
        _PROG_CACHE[key] = nc
    return _PROG_CACHE[key]


def _to_bf16_u16(a):
    """Round-to-nearest-even fp32 -> bf16 bit pattern (uint16)."""
    u = np.ascontiguousarray(a, dtype=np.float32).view(np.uint32)
    return ((u + 0x7FFF + ((u >> 16) & 1)) >> 16).astype(np.uint16)


def make_in_maps(encoder_output, decoder_hidden, coverage, Wh, bh, Ws, bs, Wc, bc,
                 v_w, v_b=None):
    f32 = np.float32
    enc = np.asarray(encoder_output, dtype=f32)
    cov = np.asarray(coverage, dtype=f32)
    Wh64 = np.asarray(Wh, dtype=np.float64)
    # u @ Wh == Wc[0] exactly (f64 solve) -> coverage folds into x
    u = np.linalg.solve(Wh64.T, np.asarray(Wc, dtype=np.float64)[0])
    A = (np.asarray(decoder_hidden, dtype=np.float64)
         @ np.asarray(Ws, dtype=np.float64)
         + np.asarray(bh, dtype=np.float64)
         + np.asarray(bs, dtype=np.float64)
         + np.asarray(bc, dtype=np.float64)).astype(f32)  # [B, H]

    vw = np.asarray(v_w, dtype=f32).reshape(HC, 128)      # chunk m -> col m
    shared = {
        "wh": _to_bf16_u16(np.asarray(Wh, dtype=f32)),
        "vwt": _to_bf16_u16(np.ascontiguousarray(vw.T)),  # [128, HC]
    }
    uf = u.astype(f32)
    in_maps = []
    for c in range(N_CORES):
        lo, hi = c * BPC, (c + 1) * BPC
        xf = enc[lo:hi] + cov[lo:hi][:, :, None] * uf
        xtc = np.ascontiguousarray(xf.transpose(0, 2, 1)).reshape(BPC * H, S)
        m = dict(shared)
        m["xt"] = _to_bf16_u16(xtc)
        m["at"] = np.ascontiguousarray(A[lo:hi].T)        # [H, bpc]
        in_maps.append(m)
    return in_maps


def run_spmd(in_maps, trace=False, **kw):
    from concourse.bass_utils import run_bass_kernel_spmd
    nc = _get_program()
    return run_bass_kernel_spmd(nc, in_maps, core_ids=list(range(N_CORES)),
                                trace=trace, **kw)


def kernel(**inputs) -> tuple[np.ndarray, np.ndarray]:
    in_maps = make_in_maps(**inputs)
    res = run_spmd(in_maps)
    a_t = np.concatenate([r["out_a"] for r in res.results], axis=0)
    a_t = a_t.astype(np.float32)
    cov = np.asarray(inputs["coverage"], dtype=np.float32)
    return a_t, cov + a_t



# revision 3
# speedup vs baseline: 1.2501x; 1.2501x over previous
"""Trainium2 (8 NeuronCores) kernel for coverage attention — v6.

Changes vs v5 (168964 ns): the H*H main matmul runs as 3 fp8-e4m3
DoubleRow passes (hi@hi + lo@hi + hi@lo split-operand scheme), which
the PE prices at 0.5 cycles/row — 1.33x fewer PE cycles than bf16 at
bf16-equivalent accuracy (emulated relerr 4.0e-3 vs 4.2e-3 bf16).
ACT tanh calls batch 2 s-groups ([128,1024] from a 2-bank PSUM tile)
with the per-partition bias A.T[ms, b], cutting ACT from 92us to 66us.
Softmax drops the max-subtraction (|e| <= ~18 so exp is safe in fp32).

Per-core pipeline, for each batch b (bpc=8):
  - DMA x~ hi/lo fp8 slabs [128p, 4kc, 2048s] (x~ = enc + cov (x) u,
    u = Wc[0] @ Wh^{-1} folded on host; pre-transposed, split into
    e4m3 hi + residual lo at scale 2^5; Wh split at 2^8)
  - for m in 4, half in 2: PSUM[128 h_m, 1024 s] accumulates 12
    DoubleRow matmuls (2 s-groups x 3 passes x 2 k-chunk-pairs);
    tanh via ACT with bias A.T[ms, b], scale 2^-13 -> f bf16
  - e-dot (deferred one batch): ps_e[1,512] += vw[ms].T @ f_m (bf16)
Epilogue: softmax without max-sub: Exp w/ accum, recip, scale.
sum_coverage = coverage + a_t on host.

Engine budget (cost model): PE 109us (96 DR-MM + 16 e-dot MM per
batch), ACT 66us, DVE ~26us, DMA ~51us.
"""

import os
import sys

for _p in ("/opt/trn_rl_repo", os.path.expanduser("~/.axon_site/_ro/trn_rl_repo")):
    if os.path.isdir(_p) and _p not in sys.path:
        sys.path.insert(0, _p)

import ml_dtypes
import numpy as np

import concourse.bass as bass
from concourse import bacc
import concourse.tile as tile
from concourse import mybir

B, S, H = 64, 2048, 512
N_CORES = 8
BPC = B // N_CORES

FP = mybir.dt.float32
BF = mybir.dt.bfloat16
F8 = mybir.dt.float8e4

SLAB = 512
NG = S // SLAB
HC = H // 128

KX = 5   # x scale exponent (max |x~| ~5.8 -> *32 = 186 < 240)
KW = 8   # Wh scale exponent (max |Wh| ~0.22 -> *256 = 56 < 240)
DR = mybir.MatmulPerfMode.DoubleRow


def build_program(bpc=BPC, s=S):
    nc = bacc.Bacc(None)

    xh = nc.declare_dram_parameter("xh", [bpc * 128, HC * s], F8, isOutput=False)
    xl = nc.declare_dram_parameter("xl", [bpc * 128, HC * s], F8, isOutput=False)
    whh = nc.declare_dram_parameter("whh", [128, HC * H], F8, isOutput=False)
    whl = nc.declare_dram_parameter("whl", [128, HC * H], F8, isOutput=False)
    at = nc.declare_dram_parameter("at", [H, bpc], FP, isOutput=False)
    vwt = nc.declare_dram_parameter("vwt", [128, HC], BF, isOutput=False)
    out_a = nc.declare_dram_parameter("out_a", [bpc, s], FP, isOutput=True)

    from contextlib import ExitStack
    with tile.TileContext(nc) as tc, ExitStack() as ctx:
        const = ctx.enter_context(tc.tile_pool(name="const", bufs=1))
        xpool = ctx.enter_context(tc.tile_pool(name="xpool", bufs=3))
        fpool = ctx.enter_context(tc.tile_pool(name="fpool", bufs=3))
        egpool = ctx.enter_context(tc.tile_pool(name="egpool", bufs=3))
        psf_pool = ctx.enter_context(tc.tile_pool(name="ps_f", bufs=3, space="PSUM"))
        pse_pool = ctx.enter_context(tc.tile_pool(name="ps_e", bufs=2, space="PSUM"))

        # ---------------- preamble ----------------
        wh_sb = {}
        for nm, src in (("h", whh), ("l", whl)):
            t = const.tile([128, HC, H], F8, tag=f"wh{nm}", name=f"wh{nm}_sb")
            nc.sync.dma_start(out=t, in_=src[:, :].rearrange("p (c j) -> p c j", c=HC))
            wh_sb[nm] = t
        at_sb = []
        for m in range(HC):
            t = const.tile([128, bpc], FP, tag=f"at{m}", name=f"at{m}")
            nc.sync.dma_start(out=t, in_=at[m * 128:(m + 1) * 128, :])
            at_sb.append(t)
        vwt_sb = const.tile([128, HC], BF, tag="vwt")
        nc.sync.dma_start(out=vwt_sb, in_=vwt[:, :])

        e_sb = const.tile([bpc, s], FP, tag="e_sb")

        # deferred e-dot matmuls: flushed one batch late so their f
        # inputs are ready when PE reaches them (no head-of-line stall)
        pend = []

        def flush(item):
            fs, bb = item
            for g in range(NG):
                ps_e = pse_pool.tile([1, SLAB], FP, tag="ps_e")
                for m in range(HC):
                    nc.tensor.matmul(
                        ps_e,
                        vwt_sb[:, m:m + 1],
                        fs[m][:, g * SLAB:(g + 1) * SLAB],
                        start=(m == 0),
                        stop=(m == HC - 1),
                    )
                e_g = egpool.tile([1, SLAB], FP, tag="e_g")
                nc.vector.tensor_copy(e_g, ps_e)
                nc.sync.dma_start(
                    out=e_sb[bb:bb + 1, g * SLAB:(g + 1) * SLAB], in_=e_g)

        # ---------------- main loop ----------------
        for b in range(bpc):
            xs = {}
            for nm, src in (("h", xh), ("l", xl)):
                t = xpool.tile([128, HC, s], F8, tag=f"xs{nm}")
                nc.sync.dma_start(
                    out=t,
                    in_=src[b * 128:(b + 1) * 128, :].rearrange(
                        "p (c ss) -> p c ss", c=HC))
                xs[nm] = t

            fs = []
            for m in range(HC):
                ms = slice(m * 128, (m + 1) * 128)
                f_m = fpool.tile([128, s], BF, tag=f"f{m}")
                for half in range(2):
                    ps = psf_pool.tile([128, 2 * SLAB], FP, tag="ps_f")
                    for gi in range(2):
                        g = half * 2 + gi
                        gs = slice(g * SLAB, (g + 1) * SLAB)
                        n = 0
                        for xa, wb in ((xs["h"], wh_sb["h"]),
                                       (xs["l"], wh_sb["h"]),
                                       (xs["h"], wh_sb["l"])):
                            for cp in range(HC // 2):
                                nc.tensor.matmul(
                                    ps[:, gi * SLAB:(gi + 1) * SLAB],
                                    wb[:, 2 * cp:2 * cp + 2, ms],
                                    xa[:, 2 * cp:2 * cp + 2, gs],
                                    start=(n == 0),
                                    stop=(n == 5),
                                    perf_mode=DR,
                                )
                                n += 1
                    nc.scalar.activation(
                        out=f_m[:, half * 2 * SLAB:(half + 1) * 2 * SLAB],
                        in_=ps,
                        func=mybir.ActivationFunctionType.Tanh,
                        bias=at_sb[m][:, b:b + 1],
                        scale=float(2.0 ** -(KX + KW)),
                    )
                fs.append(f_m)
            pend.append((fs, b))
            if len(pend) > 1:
                flush(pend.pop(0))
        while pend:
            flush(pend.pop(0))

        # ---------------- softmax + output ----------------
        # |e| <= sum|vw| ~ 18 so exp() is safe in fp32 without max-sub
        p_sb = const.tile([bpc, s], FP, tag="p_sb")
        esum = const.tile([bpc, 1], FP, tag="esum")
        nc.scalar.activation(
            out=p_sb, in_=e_sb, func=mybir.ActivationFunctionType.Exp,
            accum_out=esum,
        )
        rsum = const.tile([bpc, 1], FP, tag="rsum")
        nc.vector.reciprocal(rsum, esum)
        a_out = const.tile([bpc, s], FP, tag="a_out")
        nc.vector.tensor_scalar_mul(a_out, p_sb, rsum)
        nc.sync.dma_start(out=out_a[:, :], in_=a_out)

    return nc


_PROG_CACHE = {}


def _get_program(key=(BPC, S)):
    if key not in _PROG_CACHE:
        nc = build_program(*key)
        nc.finalize()
        _PROG_CACHE[key] = nc
    return _PROG_CACHE[key]


E4 = ml_dtypes.float8_e4m3


def _q8(v, k):
    """RNE-quantize v*2^k to TRN e4m3 (max +-240); returns float32 array
    still in the scaled domain plus the uint8 bit pattern."""
    s = np.float32(2.0 ** k)
    q = np.clip(v * s, -240.0, 240.0).astype(E4)
    return q.astype(np.float32), q.view(np.uint8)


def make_in_maps(encoder_output, decoder_hidden, coverage, Wh, bh, Ws, bs, Wc, bc,
                 v_w, v_b=None):
    f32 = np.float32
    enc = np.asarray(encoder_output, dtype=f32)
    cov = np.asarray(coverage, dtype=f32)
    Wh64 = np.asarray(Wh, dtype=np.float64)
    # u @ Wh == Wc[0] exactly (f64 solve) -> coverage folds into x
    u = np.linalg.solve(Wh64.T, np.asarray(Wc, dtype=np.float64)[0])
    A = (np.asarray(decoder_hidden, dtype=np.float64)
         @ np.asarray(Ws, dtype=np.float64)
         + np.asarray(bh, dtype=np.float64)
         + np.asarray(bs, dtype=np.float64)
         + np.asarray(bc, dtype=np.float64)).astype(f32)  # [B, H]

    Whf = np.asarray(Wh, dtype=f32)
    whh_f, whh_u8 = _q8(Whf, KW)
    whl_f, whl_u8 = _q8(Whf - whh_f / np.float32(2.0 ** KW), KW)

    def chunked(a2d):
        # [128p, 4c * N] layout with [p, c, n] = a2d[c*128+p, n]
        n = a2d.shape[1]
        return np.ascontiguousarray(
            a2d.reshape(HC, 128, n).transpose(1, 0, 2)).reshape(128, HC * n)

    vw = np.asarray(v_w, dtype=f32).reshape(HC, 128)
    shared = {
        "whh": chunked(whh_u8),
        "whl": chunked(whl_u8),
        "vwt": np.ascontiguousarray(vw.T).astype(ml_dtypes.bfloat16).view(np.uint16),
    }
    uf = u.astype(f32)
    in_maps = []
    for c in range(N_CORES):
        lo, hi = c * BPC, (c + 1) * BPC
        xf = enc[lo:hi] + cov[lo:hi][:, :, None] * uf          # [bpc, S, H]
        xt = np.ascontiguousarray(xf.transpose(0, 2, 1))       # [bpc, H, S]
        xh_f, xh_u8 = _q8(xt, KX)
        _, xl_u8 = _q8(xt - xh_f / np.float32(2.0 ** KX), KX)

        def xpack(u8):
            # [bpc*128, 4*S] with row b*128+p holding [c, s] = x[c*128+p, s]
            return np.ascontiguousarray(
                u8.reshape(BPC, HC, 128, S).transpose(0, 2, 1, 3)
            ).reshape(BPC * 128, HC * S)

        m = dict(shared)
        m["xh"] = xpack(xh_u8)
        m["xl"] = xpack(xl_u8)
        m["at"] = np.ascontiguousarray(A[lo:hi].T)             # [H, bpc]
        in_maps.append(m)
    return in_maps


def run_spmd(in_maps, trace=False, **kw):
    from concourse.bass_utils import run_bass_kernel_spmd
    nc = _get_program()
    return run_bass_kernel_spmd(nc, in_maps, core_ids=list(range(N_CORES)),
                                trace=trace, **kw)


def kernel(**inputs) -> tuple[np.ndarray, np.ndarray]:
    in_maps = make_in_maps(**inputs)
    res = run_spmd(in_maps)
    a_t = np.concatenate([r["out_a"] for r in res.results], axis=0)
    a_t = a_t.astype(np.float32)
    cov = np.asarray(inputs["coverage"], dtype=np.float32)
    return a_t, cov + a_t


# revision 7
# speedup vs baseline: 1.5236x; 1.2188x over previous
"""Trainium2 (8 NeuronCores) kernel for coverage attention — v7.

vs v6 (135161 ns): the e-dot (vw . tanh) leaves the PE entirely —
DVE does per-partition vw multiplies + bf16 tree-add, GPSIMD does the
fp32 cross-partition reduction (partition_all_reduce), freeing 27us
of PE. x DMAs split per (hi/lo, s-half) so batch-0 compute starts
~5us earlier; ~30 dummy DoubleRow matmuls on the Wh tile warm the PE
p-state while the first x slab loads; softmax for batches 0..6 runs
early (only batch 7's chain is a serial tail).

Per-core pipeline, for each batch b (bpc=8):
  - DMA x~ hi/lo fp8 slabs [128p, 4kc, 1024s] x2 (x~ = enc + cov (x) u
    folded on host, u = Wc[0] @ Wh^{-1}; e4m3 hi at 2^5 + residual lo;
    Wh split likewise at 2^8)
  - for m in 4, half in 2: PSUM[128 h_m, 1024 s] accumulates 12
    DoubleRow matmuls (2 s-groups x 3 passes x 2 k-chunk-pairs);
    tanh via ACT, bias A.T[ms, b], scale 2^-13 -> f bf16
  - e-path (deferred one batch): g_m = f_m * vw_m (DVE tensor_scalar,
    bf16); tree-add g0+=g1, g2+=g3, g0+=g2; partition_all_reduce
    (GPSIMD, fp32) -> row-copy into e_sb[b]
Epilogue: softmax without max-sub (|e| <= ~18): Exp w/ accum, recip,
scale.  sum_coverage = coverage + a_t on host.

Engine budget (cost model): PE 82us (96 DR-MM/batch), ACT 70us,
DVE ~57us, DMA ~51us, GPSIMD 24us.
"""

import os
import sys

for _p in ("/opt/trn_rl_repo", os.path.expanduser("~/.axon_site/_ro/trn_rl_repo")):
    if os.path.isdir(_p) and _p not in sys.path:
        sys.path.insert(0, _p)

import ml_dtypes
import numpy as np

import concourse.bass as bass
from concourse import bacc
from concourse import bass_isa
import concourse.tile as tile
from concourse import mybir

B, S, H = 64, 2048, 512
N_CORES = 8
BPC = B // N_CORES

FP = mybir.dt.float32
BF = mybir.dt.bfloat16
F8 = mybir.dt.float8e4

SLAB = 512
NG = S // SLAB
HC = H // 128

KX = 5   # x scale exponent (max |x~| ~5.8 -> *32 = 186 < 240)
KW = 8   # Wh scale exponent (max |Wh| ~0.22 -> *256 = 56 < 240)
DR = mybir.MatmulPerfMode.DoubleRow
N_WARM = 30


def build_program(bpc=BPC, s=S):
    nc = bacc.Bacc(None)

    xh = nc.declare_dram_parameter("xh", [bpc * 128, HC * s], F8, isOutput=False)
    xl = nc.declare_dram_parameter("xl", [bpc * 128, HC * s], F8, isOutput=False)
    whh = nc.declare_dram_parameter("whh", [128, HC * H], F8, isOutput=False)
    whl = nc.declare_dram_parameter("whl", [128, HC * H], F8, isOutput=False)
    at = nc.declare_dram_parameter("at", [H, bpc], FP, isOutput=False)
    vwt = nc.declare_dram_parameter("vwt", [128, HC], FP, isOutput=False)
    out_a = nc.declare_dram_parameter("out_a", [bpc, s], FP, isOutput=True)

    from contextlib import ExitStack
    with tile.TileContext(nc) as tc, ExitStack() as ctx:
        const = ctx.enter_context(tc.tile_pool(name="const", bufs=1))
        xpool = ctx.enter_context(tc.tile_pool(name="xpool", bufs=3))
        fpool = ctx.enter_context(tc.tile_pool(name="fpool", bufs=3))
        gpool = ctx.enter_context(tc.tile_pool(name="gpool", bufs=2))
        epool = ctx.enter_context(tc.tile_pool(name="epool", bufs=2))
        psf_pool = ctx.enter_context(tc.tile_pool(name="ps_f", bufs=3, space="PSUM"))
        warm_pool = ctx.enter_context(tc.tile_pool(name="warm", bufs=1, space="PSUM"))

        # ---------------- preamble ----------------
        wh_sb = {}
        for nm, src in (("h", whh), ("l", whl)):
            t = const.tile([128, HC, H], F8, tag=f"wh{nm}", name=f"wh{nm}_sb")
            nc.sync.dma_start(out=t, in_=src[:, :].rearrange("p (c j) -> p c j", c=HC))
            wh_sb[nm] = t
        at_sb = []
        for m in range(HC):
            t = const.tile([128, bpc], FP, tag=f"at{m}", name=f"at{m}")
            nc.sync.dma_start(out=t, in_=at[m * 128:(m + 1) * 128, :])
            at_sb.append(t)
        vwt_sb = const.tile([128, HC], FP, tag="vwt")
        nc.sync.dma_start(out=vwt_sb, in_=vwt[:, :])

        e_sb = const.tile([bpc, s], FP, tag="e_sb")

        # warm the PE p-state while the first x slab loads: ~30 dummy
        # DoubleRow matmuls against the (already loaded) Wh tile
        warm_ps = warm_pool.tile([128, SLAB], FP, tag="warm")
        for _ in range(N_WARM):
            nc.tensor.matmul(
                warm_ps,
                wh_sb["h"][:, 0:2, 0:128],
                wh_sb["h"][:, 0:2, 0:SLAB],
                start=True, stop=True, perf_mode=DR,
            )

        # deferred e-path: flushed one batch late so its f inputs are
        # ready when DVE/GPSIMD reach them
        pend = []

        def flush(item):
            fs, bb = item
            gs = []
            for m in range(HC):
                g = gpool.tile([128, s], BF, tag=f"g{m}")
                nc.vector.tensor_scalar_mul(g, fs[m], vwt_sb[:, m:m + 1])
                gs.append(g)
            nc.vector.tensor_add(gs[0], gs[0], gs[1])
            nc.vector.tensor_add(gs[2], gs[2], gs[3])
            nc.vector.tensor_add(gs[0], gs[0], gs[2])
            er = epool.tile([128, s], FP, tag="er")
            nc.gpsimd.partition_all_reduce(er, gs[0], 128, bass_isa.ReduceOp.add)
            # all-reduce output is replicated across partitions: row-DMA
            # partition 0 into e_sb[b] (engine ops can't start at partition b)
            nc.sync.dma_start(out=e_sb[bb:bb + 1, :], in_=er[0:1, :])

        # ---------------- main loop ----------------
        for b in range(bpc):
            xs = {}
            for nm, src in (("h", xh), ("l", xl)):
                t = xpool.tile([128, HC, s], F8, tag=f"xs{nm}")
                for hf in range(2):
                    nc.sync.dma_start(
                        out=t[:, :, hf * (s // 2):(hf + 1) * (s // 2)],
                        in_=src[b * 128:(b + 1) * 128, :].rearrange(
                            "p (c ss) -> p c ss", c=HC)[:, :, hf * (s // 2):(hf + 1) * (s // 2)])
                xs[nm] = t

            fs = []
            for m in range(HC):
                ms = slice(m * 128, (m + 1) * 128)
                f_m = fpool.tile([128, s], BF, tag=f"f{m}")
                for half in range(2):
                    ps = psf_pool.tile([128, 2 * SLAB], FP, tag="ps_f")
                    for gi in range(2):
                        g = half * 2 + gi
                        gsl = slice(g * SLAB, (g + 1) * SLAB)
                        n = 0
                        for xa, wb in ((xs["h"], wh_sb["h"]),
                                       (xs["l"], wh_sb["h"]),
                                       (xs["h"], wh_sb["l"])):
                            for cp in range(HC // 2):
                                nc.tensor.matmul(
                                    ps[:, gi * SLAB:(gi + 1) * SLAB],
                                    wb[:, 2 * cp:2 * cp + 2, ms],
                                    xa[:, 2 * cp:2 * cp + 2, gsl],
                                    start=(n == 0),
                                    stop=(n == 5),
                                    perf_mode=DR,
                                )
                                n += 1
                    nc.scalar.activation(
                        out=f_m[:, half * 2 * SLAB:(half + 1) * 2 * SLAB],
                        in_=ps,
                        func=mybir.ActivationFunctionType.Tanh,
                        bias=at_sb[m][:, b:b + 1],
                        scale=float(2.0 ** -(KX + KW)),
                    )
                fs.append(f_m)
            pend.append((fs, b))
            if len(pend) > 1:
                flush(pend.pop(0))

        # softmax for batches 0..6 overlaps batch 7's e-path
        # (|e| <= sum|vw| ~ 18 so exp() is safe in fp32 without max-sub)
        p_sb = const.tile([bpc, s], FP, tag="p_sb")
        esum = const.tile([bpc, 1], FP, tag="esum")
        rsum = const.tile([bpc, 1], FP, tag="rsum")
        a_out = const.tile([bpc, s], FP, tag="a_out")
        nc.scalar.activation(
            out=p_sb[0:bpc - 1, :], in_=e_sb[0:bpc - 1, :],
            func=mybir.ActivationFunctionType.Exp,
            accum_out=esum[0:bpc - 1, :],
        )
        nc.vector.reciprocal(rsum[0:bpc - 1, :], esum[0:bpc - 1, :])
        nc.vector.tensor_scalar_mul(
            a_out[0:bpc - 1, :], p_sb[0:bpc - 1, :], rsum[0:bpc - 1, :])
        nc.sync.dma_start(out=out_a[0:bpc - 1, :], in_=a_out[0:bpc - 1, :])

        while pend:
            flush(pend.pop(0))
        # engine APs must start at partition 0, so batch 7's pass redoes
        # all rows (lanes are parallel — same cost); only row 7 is new
        bl = bpc - 1
        p2 = const.tile([bpc, s], FP, tag="p2")
        esum2 = const.tile([bpc, 1], FP, tag="esum2")
        rsum2 = const.tile([bpc, 1], FP, tag="rsum2")
        a2 = const.tile([bpc, s], FP, tag="a2")
        nc.scalar.activation(
            out=p2, in_=e_sb, func=mybir.ActivationFunctionType.Exp,
            accum_out=esum2,
        )
        nc.vector.reciprocal(rsum2, esum2)
        nc.vector.tensor_scalar_mul(a2, p2, rsum2)
        nc.sync.dma_start(out=out_a[bl:bpc, :], in_=a2[bl:bpc, :])

    return nc


_PROG_CACHE = {}


def _get_program(key=(BPC, S)):
    if key not in _PROG_CACHE:
        nc = build_program(*key)
        nc.finalize()
        _PROG_CACHE[key] = nc
    return _PROG_CACHE[key]


E4 = ml_dtypes.float8_e4m3


def _q8(v, k):
    """RNE-quantize v*2^k to TRN e4m3 (max +-240); returns float32 array
    still in the scaled domain plus the uint8 bit pattern."""
    s = np.float32(2.0 ** k)
    q = np.clip(v * s, -240.0, 240.0).astype(E4)
    return q.astype(np.float32), q.view(np.uint8)


def make_in_maps(encoder_output, decoder_hidden, coverage, Wh, bh, Ws, bs, Wc, bc,
                 v_w, v_b=None):
    f32 = np.float32
    enc = np.asarray(encoder_output, dtype=f32)
    cov = np.asarray(coverage, dtype=f32)
    Wh64 = np.asarray(Wh, dtype=np.float64)
    # u @ Wh == Wc[0] exactly (f64 solve) -> coverage folds into x
    u = np.linalg.solve(Wh64.T, np.asarray(Wc, dtype=np.float64)[0])
    A = (np.asarray(decoder_hidden, dtype=np.float64)
         @ np.asarray(Ws, dtype=np.float64)
         + np.asarray(bh, dtype=np.float64)
         + np.asarray(bs, dtype=np.float64)
         + np.asarray(bc, dtype=np.float64)).astype(f32)  # [B, H]

    Whf = np.asarray(Wh, dtype=f32)
    whh_f, whh_u8 = _q8(Whf, KW)
    whl_f, whl_u8 = _q8(Whf - whh_f / np.float32(2.0 ** KW), KW)

    def chunked(a2d):
        # [128p, 4c * N] layout with [p, c, n] = a2d[c*128+p, n]
        n = a2d.shape[1]
        return np.ascontiguousarray(
            a2d.reshape(HC, 128, n).transpose(1, 0, 2)).reshape(128, HC * n)

    vw = np.asarray(v_w, dtype=f32).reshape(HC, 128)
    shared = {
        "whh": chunked(whh_u8),
        "whl": chunked(whl_u8),
        "vwt": np.ascontiguousarray(vw.T),
    }
    uf = u.astype(f32)
    in_maps = []
    for c in range(N_CORES):
        lo, hi = c * BPC, (c + 1) * BPC
        xf = enc[lo:hi] + cov[lo:hi][:, :, None] * uf          # [bpc, S, H]
        xt = np.ascontiguousarray(xf.transpose(0, 2, 1))       # [bpc, H, S]
        xh_f, xh_u8 = _q8(xt, KX)
        _, xl_u8 = _q8(xt - xh_f / np.float32(2.0 ** KX), KX)

        def xpack(u8):
            # [bpc*128, 4*S] with row b*128+p holding [c, s] = x[c*128+p, s]
            return np.ascontiguousarray(
                u8.reshape(BPC, HC, 128, S).transpose(0, 2, 1, 3)
            ).reshape(BPC * 128, HC * S)

        m = dict(shared)
        m["xh"] = xpack(xh_u8)
        m["xl"] = xpack(xl_u8)
        m["at"] = np.ascontiguousarray(A[lo:hi].T)             # [H, bpc]
        in_maps.append(m)
    return in_maps


def run_spmd(in_maps, trace=False, **kw):
    from concourse.bass_utils import run_bass_kernel_spmd
    nc = _get_program()
    return run_bass_kernel_spmd(nc, in_maps, core_ids=list(range(N_CORES)),
                                trace=trace, **kw)


def kernel(**inputs) -> tuple[np.ndarray, np.ndarray]:
    in_maps = make_in_maps(**inputs)
    res = run_spmd(in_maps)
    a_t = np.concatenate([r["out_a"] for r in res.results], axis=0)
    a_t = a_t.astype(np.float32)
    cov = np.asarray(inputs["coverage"], dtype=np.float32)
    return a_t, cov + a_t


# revision 9
# speedup vs baseline: 1.5914x; 1.0444x over previous
"""Trainium2 (8 NeuronCores) kernel for coverage attention — v8.

vs v7 (110895 ns): kills the 15.7us serial tail and early stalls.
Loops go half-outer/m-inner so each s-half's e-chain (DVE mults +
tree-add, GPSIMD partition_all_reduce) launches mid-batch instead of
one batch deferred; batch 7's softmax runs directly off the replicated
all-reduce output on partition 0 (per-half exp/mul/DMA, no row
gather); e rows ride fp16 (halves the row-gather DMA); batch 0's x
arrives in per-group slabs so real matmuls start at ~3us.

Per-core pipeline, for each batch b (bpc=8), s-half h (1024):
  - DMA x~ hi/lo fp8 slabs (x~ = enc + cov (x) u folded on host,
    u = Wc[0] @ Wh^{-1}; e4m3 hi at 2^5 + residual lo; Wh split at 2^8)
  - for m in 4: PSUM[128 h_m, 1024 s] accumulates 12 DoubleRow
    matmuls (2 s-groups x 3 passes x 2 k-chunk-pairs); tanh via ACT,
    bias A.T[ms, b], scale 2^-13 -> f bf16; DVE g_m = f_m * vw_m
  - tree-add g0+=g1, g2+=g3, g0+=g2 (DVE bf16);
    partition_all_reduce -> er fp16 (replicated);
    b<7: row-DMA er[0] -> e_sb[b, half]; b=7: exp from er[0] + accum
Epilogue: batches 0..6 batched softmax (no max-sub; |e| <= ~18);
batch 7 per-half mul + out DMA.  sum_coverage = cov + a_t on host.

Engine budget (cost model): PE ~85us (96 DR-MM/batch + warmup),
ACT ~74us, DVE ~53us, DMA ~65us, GPSIMD 24us.
"""

import os
import sys

for _p in ("/opt/trn_rl_repo", os.path.expanduser("~/.axon_site/_ro/trn_rl_repo")):
    if os.path.isdir(_p) and _p not in sys.path:
        sys.path.insert(0, _p)

import ml_dtypes
import numpy as np

import concourse.bass as bass
from concourse import bacc
from concourse import bass_isa
import concourse.tile as tile
from concourse import mybir

B, S, H = 64, 2048, 512
N_CORES = 8
BPC = B // N_CORES

FP = mybir.dt.float32
F16 = mybir.dt.float16
BF = mybir.dt.bfloat16
F8 = mybir.dt.float8e4

SLAB = 512
NG = S // SLAB
HC = H // 128
HS = S // 2

KX = 5   # x scale exponent (max |x~| ~5.8 -> *32 = 186 < 240)
KW = 8   # Wh scale exponent (max |Wh| ~0.22 -> *256 = 56 < 240)
DR = mybir.MatmulPerfMode.DoubleRow
N_WARM = 10


def build_program(bpc=BPC, s=S):
    nc = bacc.Bacc(None)

    xh = nc.declare_dram_parameter("xh", [bpc * 128, HC * s], F8, isOutput=False)
    xl = nc.declare_dram_parameter("xl", [bpc * 128, HC * s], F8, isOutput=False)
    whh = nc.declare_dram_parameter("whh", [128, HC * H], F8, isOutput=False)
    whl = nc.declare_dram_parameter("whl", [128, HC * H], F8, isOutput=False)
    at = nc.declare_dram_parameter("at", [H, bpc], FP, isOutput=False)
    vwt = nc.declare_dram_parameter("vwt", [128, HC], FP, isOutput=False)
    out_a = nc.declare_dram_parameter("out_a", [bpc, s], FP, isOutput=True)

    from contextlib import ExitStack
    with tile.TileContext(nc) as tc, ExitStack() as ctx:
        const = ctx.enter_context(tc.tile_pool(name="const", bufs=1))
        xpool = ctx.enter_context(tc.tile_pool(name="xpool", bufs=3))
        fpool = ctx.enter_context(tc.tile_pool(name="fpool", bufs=2))
        gpool = ctx.enter_context(tc.tile_pool(name="gpool", bufs=2))
        epool = ctx.enter_context(tc.tile_pool(name="epool", bufs=2))
        psf_pool = ctx.enter_context(tc.tile_pool(name="ps_f", bufs=3, space="PSUM"))
        warm_pool = ctx.enter_context(tc.tile_pool(name="warm", bufs=1, space="PSUM"))

        # ---------------- preamble ----------------
        wh_sb = {}
        for nm, src in (("h", whh), ("l", whl)):
            t = const.tile([128, HC, H], F8, tag=f"wh{nm}", name=f"wh{nm}_sb")
            nc.sync.dma_start(out=t, in_=src[:, :].rearrange("p (c j) -> p c j", c=HC))
            wh_sb[nm] = t
        at_sb = []
        for m in range(HC):
            t = const.tile([128, bpc], FP, tag=f"at{m}", name=f"at{m}")
            nc.sync.dma_start(out=t, in_=at[m * 128:(m + 1) * 128, :])
            at_sb.append(t)
        vwt_sb = const.tile([128, HC], FP, tag="vwt")
        nc.sync.dma_start(out=vwt_sb, in_=vwt[:, :])

        e_sb = const.tile([bpc, s], F16, tag="e_sb")

        # warm the PE p-state while the first x slab loads
        warm_ps = warm_pool.tile([128, SLAB], FP, tag="warm")
        for _ in range(N_WARM):
            nc.tensor.matmul(
                warm_ps,
                wh_sb["h"][:, 0:2, 0:128],
                wh_sb["h"][:, 0:2, 0:SLAB],
                start=True, stop=True, perf_mode=DR,
            )

        # batch-7 per-half softmax scratch (all on partition 0)
        p7 = [const.tile([1, HS], FP, tag=f"p7{hf}", name=f"p7{hf}") for hf in range(2)]
        s7 = [const.tile([1, 1], FP, tag=f"s7{hf}", name=f"s7{hf}") for hf in range(2)]
        ssum = const.tile([1, 1], FP, tag="ssum")
        rs7 = const.tile([1, 1], FP, tag="rs7")
        a7 = [const.tile([1, HS], FP, tag=f"a7{hf}", name=f"a7{hf}") for hf in range(2)]

        # ---------------- main loop ----------------
        for b in range(bpc):
            last = b == bpc - 1
            if last:
                # batches 0..6: batched softmax overlapping batch 7
                p_sb = const.tile([bpc, s], FP, tag="p_sb")
                esum = const.tile([bpc, 1], FP, tag="esum")
                rsum = const.tile([bpc, 1], FP, tag="rsum")
                a_out = const.tile([bpc, s], FP, tag="a_out")
                nc.scalar.activation(
                    out=p_sb[0:bpc - 1, :], in_=e_sb[0:bpc - 1, :],
                    func=mybir.ActivationFunctionType.Exp,
                    accum_out=esum[0:bpc - 1, :],
                )
                nc.vector.reciprocal(rsum[0:bpc - 1, :], esum[0:bpc - 1, :])
                nc.vector.tensor_scalar_mul(
                    a_out[0:bpc - 1, :], p_sb[0:bpc - 1, :], rsum[0:bpc - 1, :])
                nc.sync.dma_start(out=out_a[0:bpc - 1, :], in_=a_out[0:bpc - 1, :])

            xs = {}
            for nm, src in (("h", xh), ("l", xl)):
                t = xpool.tile([128, HC, s], F8, tag=f"xs{nm}")
                src_r = src[b * 128:(b + 1) * 128, :].rearrange(
                    "p (c ss) -> p c ss", c=HC)
                npc = 4 if b == 0 else 2   # finer slabs for batch 0
                for pc in range(npc):
                    sl = slice(pc * (s // npc), (pc + 1) * (s // npc))
                    nc.sync.dma_start(out=t[:, :, sl], in_=src_r[:, :, sl])
                xs[nm] = t

            for half in range(2):
                hsl = slice(half * HS, (half + 1) * HS)
                gs = []
                for m in range(HC):
                    ms = slice(m * 128, (m + 1) * 128)
                    f_m = fpool.tile([128, HS], BF, tag=f"f{m}")
                    ps = psf_pool.tile([128, HS], FP, tag="ps_f")
                    for gi in range(2):
                        g = half * 2 + gi
                        gsl = slice(g * SLAB, (g + 1) * SLAB)
                        n = 0
                        for xa, wb in ((xs["h"], wh_sb["h"]),
                                       (xs["l"], wh_sb["h"]),
                                       (xs["h"], wh_sb["l"])):
                            for cp in range(HC // 2):
                                nc.tensor.matmul(
                                    ps[:, gi * SLAB:(gi + 1) * SLAB],
                                    wb[:, 2 * cp:2 * cp + 2, ms],
                                    xa[:, 2 * cp:2 * cp + 2, gsl],
                                    start=(n == 0),
                                    stop=(n == 5),
                                    perf_mode=DR,
                                )
                                n += 1
                    nc.scalar.activation(
                        out=f_m, in_=ps,
                        func=mybir.ActivationFunctionType.Tanh,
                        bias=at_sb[m][:, b:b + 1],
                        scale=float(2.0 ** -(KX + KW)),
                    )
                    g_m = gpool.tile([128, HS], BF, tag=f"g{m}")
                    nc.vector.tensor_scalar_mul(g_m, f_m, vwt_sb[:, m:m + 1])
                    gs.append(g_m)

                nc.vector.tensor_add(gs[0], gs[0], gs[1])
                nc.vector.tensor_add(gs[2], gs[2], gs[3])
                nc.vector.tensor_add(gs[0], gs[0], gs[2])
                er = epool.tile([128, HS], F16, tag="er")
                nc.gpsimd.partition_all_reduce(
                    er, gs[0], 128, bass_isa.ReduceOp.add)
                if not last:
                    # all-reduce output is replicated: row-DMA partition 0
                    nc.sync.dma_start(out=e_sb[b:b + 1, hsl], in_=er[0:1, :])
                else:
                    # batch 7: softmax pieces straight off partition 0
                    nc.scalar.activation(
                        out=p7[half], in_=er[0:1, :],
                        func=mybir.ActivationFunctionType.Exp,
                        accum_out=s7[half],
                    )

        # batch-7 epilogue: combine halves, normalize, write out
        nc.vector.tensor_add(ssum, s7[0], s7[1])
        nc.vector.reciprocal(rs7, ssum)
        for hf in range(2):
            nc.vector.tensor_scalar_mul(a7[hf], p7[hf], rs7)
            nc.sync.dma_start(
                out=out_a[bpc - 1:bpc, hf * HS:(hf + 1) * HS], in_=a7[hf])

    return nc


_PROG_CACHE = {}


def _get_program(key=(BPC, S)):
    if key not in _PROG_CACHE:
        nc = build_program(*key)
        nc.finalize()
        _PROG_CACHE[key] = nc
    return _PROG_CACHE[key]


E4 = ml_dtypes.float8_e4m3


def _q8(v, k):
    """RNE-quantize v*2^k to TRN e4m3 (max +-240); returns float32 array
    still in the scaled domain plus the uint8 bit pattern."""
    s = np.float32(2.0 ** k)
    q = np.clip(v * s, -240.0, 240.0).astype(E4)
    return q.astype(np.float32), q.view(np.uint8)


def make_in_maps(encoder_output, decoder_hidden, coverage, Wh, bh, Ws, bs, Wc, bc,
                 v_w, v_b=None):
    f32 = np.float32
    enc = np.asarray(encoder_output, dtype=f32)
    cov = np.asarray(coverage, dtype=f32)
    Wh64 = np.asarray(Wh, dtype=np.float64)
    # u @ Wh == Wc[0] exactly (f64 solve) -> coverage folds into x
    u = np.linalg.solve(Wh64.T, np.asarray(Wc, dtype=np.float64)[0])
    A = (np.asarray(decoder_hidden, dtype=np.float64)
         @ np.asarray(Ws, dtype=np.float64)
         + np.asarray(bh, dtype=np.float64)
         + np.asarray(bs, dtype=np.float64)
         + np.asarray(bc, dtype=np.float64)).astype(f32)  # [B, H]

    Whf = np.asarray(Wh, dtype=f32)
    whh_f, whh_u8 = _q8(Whf, KW)
    whl_f, whl_u8 = _q8(Whf - whh_f / np.float32(2.0 ** KW), KW)

    def chunked(a2d):
        # [128p, 4c * N] layout with [p, c, n] = a2d[c*128+p, n]
        n = a2d.shape[1]
        return np.ascontiguousarray(
            a2d.reshape(HC, 128, n).transpose(1, 0, 2)).reshape(128, HC * n)

    vw = np.asarray(v_w, dtype=f32).reshape(HC, 128)
    shared = {
        "whh": chunked(whh_u8),
        "whl": chunked(whl_u8),
        "vwt": np.ascontiguousarray(vw.T),
    }
    uf = u.astype(f32)
    in_maps = []
    for c in range(N_CORES):
        lo, hi = c * BPC, (c + 1) * BPC
        xf = enc[lo:hi] + cov[lo:hi][:, :, None] * uf          # [bpc, S, H]
        xt = np.ascontiguousarray(xf.transpose(0, 2, 1))       # [bpc, H, S]
        xh_f, xh_u8 = _q8(xt, KX)
        _, xl_u8 = _q8(xt - xh_f / np.float32(2.0 ** KX), KX)

        def xpack(u8):
            # [bpc*128, 4*S] with row b*128+p holding [c, s] = x[c*128+p, s]
            return np.ascontiguousarray(
                u8.reshape(BPC, HC, 128, S).transpose(0, 2, 1, 3)
            ).reshape(BPC * 128, HC * S)

        m = dict(shared)
        m["xh"] = xpack(xh_u8)
        m["xl"] = xpack(xl_u8)
        m["at"] = np.ascontiguousarray(A[lo:hi].T)             # [H, bpc]
        in_maps.append(m)
    return in_maps


def run_spmd(in_maps, trace=False, **kw):
    from concourse.bass_utils import run_bass_kernel_spmd
    nc = _get_program()
    return run_bass_kernel_spmd(nc, in_maps, core_ids=list(range(N_CORES)),
                                trace=trace, **kw)


def kernel(**inputs) -> tuple[np.ndarray, np.ndarray]:
    in_maps = make_in_maps(**inputs)
    res = run_spmd(in_maps)
    a_t = np.concatenate([r["out_a"] for r in res.results], axis=0)
    a_t = a_t.astype(np.float32)
    cov = np.asarray(inputs["coverage"], dtype=np.float32)
    return a_t, cov + a_t


# revision 18
# speedup vs baseline: 1.6003x; 1.0056x over previous
"""Trainium2 (8 NeuronCores) kernel for coverage attention — v9.

vs v8 (106176 ns): e-chain uses fused scalar_tensor_tensor
(g = f_m*vw_m + g, one DVE op per m instead of mult+add trees);
batch 7 runs an uneven s-split (1536/512) so the final serial
e-chain/softmax tail covers only 512 columns; a7 is one tile ->
one output DMA.

Per-core pipeline, for each batch b (bpc=8), s-piece (1024+1024,
last batch 1536+512):
  - DMA x~ hi/lo fp8 slabs (x~ = enc + cov (x) u folded on host,
    u = Wc[0] @ Wh^{-1}; e4m3 hi at 2^5 + residual lo; Wh split at 2^8)
  - for m in 4: PSUM[128 h_m, L s] accumulates 6*L/512 DoubleRow
    matmuls (3 passes x 2 k-chunk-pairs per 512-group); tanh via ACT,
    bias A.T[ms, b], scale 2^-13 -> f bf16; DVE g = f_m*vw_m (+ g)
  - partition_all_reduce -> er fp16 (replicated);
    b<7: row-DMA er[0] -> e_sb[b, piece]; b=7: exp from er[0] + accum
Epilogue: batches 0..6 batched softmax (no max-sub; |e| <= ~18);
batch 7 per-piece mul into one a7 tile + one DMA.
sum_coverage = cov + a_t on host.
"""

import os
import sys

for _p in ("/opt/trn_rl_repo", os.path.expanduser("~/.axon_site/_ro/trn_rl_repo")):
    if os.path.isdir(_p) and _p not in sys.path:
        sys.path.insert(0, _p)

import ml_dtypes
import numpy as np

import concourse.bass as bass
from concourse import bacc
from concourse import bass_isa
import concourse.tile as tile
from concourse import mybir

B, S, H = 64, 2048, 512
N_CORES = 8
BPC = B // N_CORES

FP = mybir.dt.float32
F16 = mybir.dt.float16
BF = mybir.dt.bfloat16
F8 = mybir.dt.float8e4

SLAB = 512
HC = H // 128

KX = 5   # x scale exponent (max |x~| ~5.8 -> *32 = 186 < 240)
KW = 8   # Wh scale exponent (max |Wh| ~0.22 -> *256 = 56 < 240)
DR = mybir.MatmulPerfMode.DoubleRow

# tuning knobs (sim-A/B'd)
N_WARM = 25
WARM_BUFS = 2
LAST_SPLIT = "1024,512,512"
PSF_BUFS = 3
B0_SLABS = 4
XPOOL_BUFS = 3
ROWDMA_GPSIMD = 0
SKIP = set()


def build_program(bpc=BPC, s=S):
    nc = bacc.Bacc(None)

    xh = nc.declare_dram_parameter("xh", [bpc * 128, HC * s], F8, isOutput=False)
    xl = nc.declare_dram_parameter("xl", [bpc * 128, HC * s], F8, isOutput=False)
    whh = nc.declare_dram_parameter("whh", [128, HC * H], F8, isOutput=False)
    whl = nc.declare_dram_parameter("whl", [128, HC * H], F8, isOutput=False)
    at = nc.declare_dram_parameter("at", [H, bpc], FP, isOutput=False)
    vwt = nc.declare_dram_parameter("vwt", [128, HC], FP, isOutput=False)
    out_a = nc.declare_dram_parameter("out_a", [bpc, s], FP, isOutput=True)

    last_split = [int(v) for v in LAST_SPLIT.split(",")]
    assert sum(last_split) == s and all(v % SLAB == 0 for v in last_split)
    max_piece = max(max(last_split), s // 2)

    from contextlib import ExitStack
    with tile.TileContext(nc) as tc, ExitStack() as ctx:
        const = ctx.enter_context(tc.tile_pool(name="const", bufs=1))
        xpool = ctx.enter_context(tc.tile_pool(name="xpool", bufs=XPOOL_BUFS))
        fpool = ctx.enter_context(tc.tile_pool(name="fpool", bufs=2))
        gpool = ctx.enter_context(tc.tile_pool(name="gpool", bufs=2))
        epool = ctx.enter_context(tc.tile_pool(name="epool", bufs=2))
        psf_pool = ctx.enter_context(
            tc.tile_pool(name="ps_f", bufs=PSF_BUFS, space="PSUM"))
        warm_pool = ctx.enter_context(
            tc.tile_pool(name="warm", bufs=WARM_BUFS, space="PSUM"))

        # ---------------- preamble ----------------
        wh_sb = {}
        for nm, src in (("h", whh), ("l", whl)):
            t = const.tile([128, HC, H], F8, tag=f"wh{nm}", name=f"wh{nm}_sb")
            nc.sync.dma_start(out=t, in_=src[:, :].rearrange("p (c j) -> p c j", c=HC))
            wh_sb[nm] = t
        at_sb = []
        for m in range(HC):
            t = const.tile([128, bpc], FP, tag=f"at{m}", name=f"at{m}")
            nc.sync.dma_start(out=t, in_=at[m * 128:(m + 1) * 128, :])
            at_sb.append(t)
        vwt_sb = const.tile([128, HC], FP, tag="vwt")
        nc.sync.dma_start(out=vwt_sb, in_=vwt[:, :])

        e_sb = const.tile([bpc, s], F16, tag="e_sb")

        # warm the PE p-state while the first x slab loads
        for i in range(N_WARM):
            wt = warm_pool.tile([128, SLAB], FP, tag="warm", name="wt")
            nc.tensor.matmul(
                wt,
                wh_sb["h"][:, 0:2, 0:128],
                wh_sb["h"][:, 0:2, 0:SLAB],
                start=True, stop=True, perf_mode=DR,
            )

        # batch-7 per-piece softmax scratch (all on partition 0)
        n_lp = len(last_split)
        p7 = [const.tile([1, max_piece], FP, tag=f"p7{i}", name=f"p7{i}")
              for i in range(n_lp)]
        s7 = [const.tile([1, 1], FP, tag=f"s7{i}", name=f"s7{i}")
              for i in range(n_lp)]
        ssum = const.tile([1, 1], FP, tag="ssum")
        rs7 = const.tile([1, 1], FP, tag="rs7")
        a7 = const.tile([1, s], FP, tag="a7")

        # ---------------- main loop ----------------
        for b in range(bpc):
            last = b == bpc - 1
            xs = {}
            for nm, src in (("h", xh), ("l", xl)):
                t = xpool.tile([128, HC, s], F8, tag=f"xs{nm}")
                src_r = src[b * 128:(b + 1) * 128, :].rearrange(
                    "p (c ss) -> p c ss", c=HC)
                npc = B0_SLABS if b == 0 else 2
                for pc in range(npc):
                    sl = slice(pc * (s // npc), (pc + 1) * (s // npc))
                    nc.sync.dma_start(out=t[:, :, sl], in_=src_r[:, :, sl])
                xs[nm] = t

            if last and "bsm" not in SKIP:
                # batches 0..6: batched softmax overlapping batch 7
                # (emitted after b7's x loads so its out-DMA doesn't block
                # the FIFO DMA queue ahead of the prefetch)
                p_sb = const.tile([bpc, s], FP, tag="p_sb")
                esum = const.tile([bpc, 1], FP, tag="esum")
                rsum = const.tile([bpc, 1], FP, tag="rsum")
                a_out = const.tile([bpc, s], FP, tag="a_out")
                nc.scalar.activation(
                    out=p_sb[0:bpc - 1, :], in_=e_sb[0:bpc - 1, :],
                    func=mybir.ActivationFunctionType.Exp,
                    accum_out=esum[0:bpc - 1, :],
                )
                nc.vector.reciprocal(rsum[0:bpc - 1, :], esum[0:bpc - 1, :])
                nc.vector.tensor_scalar_mul(
                    a_out[0:bpc - 1, :], p_sb[0:bpc - 1, :], rsum[0:bpc - 1, :])
                nc.sync.dma_start(out=out_a[0:bpc - 1, :], in_=a_out[0:bpc - 1, :])

            pieces = last_split if last else [s // 2, s // 2]
            pstart = 0
            for pi, plen in enumerate(pieces):
                g_acc = gpool.tile([128, max_piece], BF, tag="g_acc")
                for m in range(HC):
                    ms = slice(m * 128, (m + 1) * 128)
                    f_m = fpool.tile([128, max_piece], BF, tag=f"f{m}")
                    ps = psf_pool.tile([128, max_piece], FP, tag="ps_f")
                    for gi in range(plen // SLAB):
                        goff = pstart + gi * SLAB
                        gsl = slice(goff, goff + SLAB)
                        n = 0
                        for xa, wb in ((xs["h"], wh_sb["h"]),
                                       (xs["l"], wh_sb["h"]),
                                       (xs["h"], wh_sb["l"])):
                            for cp in range(HC // 2):
                                nc.tensor.matmul(
                                    ps[:, gi * SLAB:(gi + 1) * SLAB],
                                    wb[:, 2 * cp:2 * cp + 2, ms],
                                    xa[:, 2 * cp:2 * cp + 2, gsl],
                                    start=(n == 0),
                                    stop=(n == 5),
                                    perf_mode=DR,
                                )
                                n += 1
                    nc.scalar.activation(
                        out=f_m[:, 0:plen], in_=ps[:, 0:plen],
                        func=mybir.ActivationFunctionType.Tanh,
                        bias=at_sb[m][:, b:b + 1],
                        scale=float(2.0 ** -(KX + KW)),
                    )
                    if m == 0:
                        nc.vector.tensor_scalar_mul(
                            g_acc[:, 0:plen], f_m[:, 0:plen], vwt_sb[:, 0:1])
                    else:
                        # g += f_m * vw_m, fused on DVE
                        nc.vector.scalar_tensor_tensor(
                            g_acc[:, 0:plen], f_m[:, 0:plen],
                            vwt_sb[:, m:m + 1], g_acc[:, 0:plen],
                            op0=mybir.AluOpType.mult,
                            op1=mybir.AluOpType.add,
                        )
                er = epool.tile([128, max_piece], F16, tag="er")
                nc.gpsimd.partition_all_reduce(
                    er[:, 0:plen], g_acc[:, 0:plen], 128, bass_isa.ReduceOp.add)
                if not last:
                    # all-reduce output is replicated: row-DMA partition 0
                    if "row" not in SKIP:
                        eng = nc.gpsimd if ROWDMA_GPSIMD else nc.sync
                        eng.dma_start(
                            out=e_sb[b:b + 1, pstart:pstart + plen],
                            in_=er[0:1, 0:plen])
                elif "b7sm" in SKIP:
                    pass
                else:
                    nc.scalar.activation(
                        out=p7[pi][:, 0:plen], in_=er[0:1, 0:plen],
                        func=mybir.ActivationFunctionType.Exp,
                        accum_out=s7[pi],
                    )
                pstart += plen

        # batch-7 epilogue: combine pieces, normalize, one output DMA
        if "b7sm" in SKIP:
            nc.sync.dma_start(out=out_a[bpc - 1:bpc, :], in_=a7)
            skip_epilogue = True
        else:
            skip_epilogue = False
        if not skip_epilogue:
            nc.vector.tensor_add(ssum, s7[0], s7[1])
        if not skip_epilogue:
            for i in range(2, n_lp):
                nc.vector.tensor_add(ssum, ssum, s7[i])
            nc.vector.reciprocal(rs7, ssum)
            pstart = 0
            for pi, plen in enumerate(last_split):
                nc.vector.tensor_scalar_mul(
                    a7[:, pstart:pstart + plen], p7[pi][:, 0:plen], rs7)
                pstart += plen
            nc.sync.dma_start(out=out_a[bpc - 1:bpc, :], in_=a7)

    return nc


_PROG_CACHE = {}


def _get_program(key=(BPC, S)):
    if key not in _PROG_CACHE:
        nc = build_program(*key)
        nc.finalize()
        _PROG_CACHE[key] = nc
    return _PROG_CACHE[key]


E4 = ml_dtypes.float8_e4m3


def _q8(v, k):
    """RNE-quantize v*2^k to TRN e4m3 (max +-240); returns float32 array
    still in the scaled domain plus the uint8 bit pattern."""
    s = np.float32(2.0 ** k)
    q = np.clip(v * s, -240.0, 240.0).astype(E4)
    return q.astype(np.float32), q.view(np.uint8)


def make_in_maps(encoder_output, decoder_hidden, coverage, Wh, bh, Ws, bs, Wc, bc,
                 v_w, v_b=None):
    f32 = np.float32
    enc = np.asarray(encoder_output, dtype=f32)
    cov = np.asarray(coverage, dtype=f32)
    Wh64 = np.asarray(Wh, dtype=np.float64)
    # u @ Wh == Wc[0] exactly (f64 solve) -> coverage folds into x
    u = np.linalg.solve(Wh64.T, np.asarray(Wc, dtype=np.float64)[0])
    A = (np.asarray(decoder_hidden, dtype=np.float64)
         @ np.asarray(Ws, dtype=np.float64)
         + np.asarray(bh, dtype=np.float64)
         + np.asarray(bs, dtype=np.float64)
         + np.asarray(bc, dtype=np.float64)).astype(f32)  # [B, H]

    Whf = np.asarray(Wh, dtype=f32)
    whh_f, whh_u8 = _q8(Whf, KW)
    whl_f, whl_u8 = _q8(Whf - whh_f / np.float32(2.0 ** KW), KW)

    def chunked(a2d):
        # [128p, 4c * N] layout with [p, c, n] = a2d[c*128+p, n]
        n = a2d.shape[1]
        return np.ascontiguousarray(
            a2d.reshape(HC, 128, n).transpose(1, 0, 2)).reshape(128, HC * n)

    vw = np.asarray(v_w, dtype=f32).reshape(HC, 128)
    shared = {
        "whh": chunked(whh_u8),
        "whl": chunked(whl_u8),
        "vwt": np.ascontiguousarray(vw.T),
    }
    uf = u.astype(f32)
    in_maps = []
    for c in range(N_CORES):
        lo, hi = c * BPC, (c + 1) * BPC
        xf = enc[lo:hi] + cov[lo:hi][:, :, None] * uf          # [bpc, S, H]
        xt = np.ascontiguousarray(xf.transpose(0, 2, 1))       # [bpc, H, S]
        xh_f, xh_u8 = _q8(xt, KX)
        _, xl_u8 = _q8(xt - xh_f / np.float32(2.0 ** KX), KX)

        def xpack(u8):
            # [bpc*128, 4*S] with row b*128+p holding [c, s] = x[c*128+p, s]
            return np.ascontiguousarray(
                u8.reshape(BPC, HC, 128, S).transpose(0, 2, 1, 3)
            ).reshape(BPC * 128, HC * S)

        m = dict(shared)
        m["xh"] = xpack(xh_u8)
        m["xl"] = xpack(xl_u8)
        m["at"] = np.ascontiguousarray(A[lo:hi].T)             # [H, bpc]
        in_maps.append(m)
    return in_maps


def run_spmd(in_maps, trace=False, **kw):
    from concourse.bass_utils import run_bass_kernel_spmd
    nc = _get_program()
    return run_bass_kernel_spmd(nc, in_maps, core_ids=list(range(N_CORES)),
                                trace=trace, **kw)


def kernel(**inputs) -> tuple[np.ndarray, np.ndarray]:
    in_maps = make_in_maps(**inputs)
    res = run_spmd(in_maps)
    a_t = np.concatenate([r["out_a"] for r in res.results], axis=0)
    a_t = a_t.astype(np.float32)
    cov = np.asarray(inputs["coverage"], dtype=np.float32)
    return a_t, cov + a_t


# revision 20
# speedup vs baseline: 1.6176x; 1.0108x over previous
"""Trainium2 (8 NeuronCores) kernel for coverage attention — v9.

vs v8 (106176 ns): e-chain uses fused scalar_tensor_tensor
(g = f_m*vw_m + g, one DVE op per m instead of mult+add trees);
batch 7 runs an uneven s-split (1536/512) so the final serial
e-chain/softmax tail covers only 512 columns; a7 is one tile ->
one output DMA.

Per-core pipeline, for each batch b (bpc=8), s-piece (1024+1024,
last batch 1536+512):
  - DMA x~ hi/lo fp8 slabs (x~ = enc + cov (x) u folded on host,
    u = Wc[0] @ Wh^{-1}; e4m3 hi at 2^5 + residual lo; Wh split at 2^8)
  - for m in 4: PSUM[128 h_m, L s] accumulates 6*L/512 DoubleRow
    matmuls (3 passes x 2 k-chunk-pairs per 512-group); tanh via ACT,
    bias A.T[ms, b], scale 2^-13 -> f bf16; DVE g = f_m*vw_m (+ g)
  - partition_all_reduce -> er fp16 (replicated);
    b<7: row-DMA er[0] -> e_sb[b, piece]; b=7: exp from er[0] + accum
Epilogue: batches 0..6 batched softmax (no max-sub; |e| <= ~18);
batch 7 per-piece mul into one a7 tile + one DMA.
sum_coverage = cov + a_t on host.
"""

import os
import sys

for _p in ("/opt/trn_rl_repo", os.path.expanduser("~/.axon_site/_ro/trn_rl_repo")):
    if os.path.isdir(_p) and _p not in sys.path:
        sys.path.insert(0, _p)

import ml_dtypes
import numpy as np

import concourse.bass as bass
from concourse import bacc
from concourse import bass_isa
import concourse.tile as tile
from concourse import mybir

B, S, H = 64, 2048, 512
N_CORES = 8
BPC = B // N_CORES

FP = mybir.dt.float32
F16 = mybir.dt.float16
BF = mybir.dt.bfloat16
F8 = mybir.dt.float8e4

SLAB = 512
HC = H // 128

KX = 5   # x scale exponent (max |x~| ~5.8 -> *32 = 186 < 240)
KW = 8   # Wh scale exponent (max |Wh| ~0.22 -> *256 = 56 < 240)
DR = mybir.MatmulPerfMode.DoubleRow

# tuning knobs (sim-A/B'd)
N_WARM = 25
WARM_BUFS = 1
LAST_SPLIT = "1024,512,512"
PSF_BUFS = 3
B0_SLABS = 4
XPOOL_BUFS = 3
ROWDMA_GPSIMD = 0
SKIP = set()


def build_program(bpc=BPC, s=S):
    nc = bacc.Bacc(None)

    xh = nc.declare_dram_parameter("xh", [bpc * 128, HC * s], F8, isOutput=False)
    xl = nc.declare_dram_parameter("xl", [bpc * 128, HC * s], F8, isOutput=False)
    whh = nc.declare_dram_parameter("whh", [128, HC * H], F8, isOutput=False)
    whl = nc.declare_dram_parameter("whl", [128, HC * H], F8, isOutput=False)
    at = nc.declare_dram_parameter("at", [H, bpc], FP, isOutput=False)
    vwt = nc.declare_dram_parameter("vwt", [128, HC], FP, isOutput=False)
    out_a = nc.declare_dram_parameter("out_a", [bpc, s], FP, isOutput=True)

    last_split = [int(v) for v in LAST_SPLIT.split(",")]
    assert sum(last_split) == s and all(v % SLAB == 0 for v in last_split)
    max_piece = max(max(last_split), s // 2)

    from contextlib import ExitStack
    with tile.TileContext(nc) as tc, ExitStack() as ctx:
        const = ctx.enter_context(tc.tile_pool(name="const", bufs=1))
        xpool = ctx.enter_context(tc.tile_pool(name="xpool", bufs=XPOOL_BUFS))
        fpool = ctx.enter_context(tc.tile_pool(name="fpool", bufs=2))
        gpool = ctx.enter_context(tc.tile_pool(name="gpool", bufs=2))
        epool = ctx.enter_context(tc.tile_pool(name="epool", bufs=2))
        psf_pool = ctx.enter_context(
            tc.tile_pool(name="ps_f", bufs=PSF_BUFS, space="PSUM"))
        warm_pool = ctx.enter_context(
            tc.tile_pool(name="warm", bufs=WARM_BUFS, space="PSUM"))

        # ---------------- preamble ----------------
        wh_sb = {}
        for nm, src in (("h", whh), ("l", whl)):
            t = const.tile([128, HC, H], F8, tag=f"wh{nm}", name=f"wh{nm}_sb")
            nc.sync.dma_start(out=t, in_=src[:, :].rearrange("p (c j) -> p c j", c=HC))
            wh_sb[nm] = t
        at_sb = []
        for m in range(HC):
            t = const.tile([128, bpc], FP, tag=f"at{m}", name=f"at{m}")
            nc.sync.dma_start(out=t, in_=at[m * 128:(m + 1) * 128, :])
            at_sb.append(t)
        vwt_sb = const.tile([128, HC], FP, tag="vwt")
        nc.sync.dma_start(out=vwt_sb, in_=vwt[:, :])
        vwb = nc.declare_dram_parameter("vwb", [128, HC], BF, isOutput=False)
        vwb_sb = const.tile([128, HC], BF, tag="vwb")
        nc.sync.dma_start(out=vwb_sb, in_=vwb[:, :])
        pse_pool = ctx.enter_context(tc.tile_pool(name="ps_e", bufs=1, space="PSUM"))

        e_sb = const.tile([bpc, s], F16, tag="e_sb")

        # warm the PE p-state while the first x slab loads
        for i in range(N_WARM):
            wt = warm_pool.tile([128, SLAB], FP, tag="warm", name="wt")
            nc.tensor.matmul(
                wt,
                wh_sb["h"][:, 0:2, 0:128],
                wh_sb["h"][:, 0:2, 0:SLAB],
                start=True, stop=True, perf_mode=DR,
            )

        # batch-7 per-piece softmax scratch (all on partition 0)
        n_lp = len(last_split)
        p7 = [const.tile([1, max_piece], FP, tag=f"p7{i}", name=f"p7{i}")
              for i in range(n_lp)]
        s7 = [const.tile([1, 1], FP, tag=f"s7{i}", name=f"s7{i}")
              for i in range(n_lp)]
        ssum = const.tile([1, 1], FP, tag="ssum")
        rs7 = const.tile([1, 1], FP, tag="rs7")
        a7 = const.tile([1, s], FP, tag="a7")

        # ---------------- main loop ----------------
        for b in range(bpc):
            last = b == bpc - 1
            xs = {}
            for nm, src in (("h", xh), ("l", xl)):
                t = xpool.tile([128, HC, s], F8, tag=f"xs{nm}")
                src_r = src[b * 128:(b + 1) * 128, :].rearrange(
                    "p (c ss) -> p c ss", c=HC)
                npc = B0_SLABS if b == 0 else 2
                for pc in range(npc):
                    sl = slice(pc * (s // npc), (pc + 1) * (s // npc))
                    nc.sync.dma_start(out=t[:, :, sl], in_=src_r[:, :, sl])
                xs[nm] = t

            if last and "bsm" not in SKIP:
                # batches 0..6: batched softmax overlapping batch 7
                # (emitted after b7's x loads so its out-DMA doesn't block
                # the FIFO DMA queue ahead of the prefetch)
                p_sb = const.tile([bpc, s], FP, tag="p_sb")
                esum = const.tile([bpc, 1], FP, tag="esum")
                rsum = const.tile([bpc, 1], FP, tag="rsum")
                a_out = const.tile([bpc, s], FP, tag="a_out")
                nc.scalar.activation(
                    out=p_sb[0:bpc - 1, :], in_=e_sb[0:bpc - 1, :],
                    func=mybir.ActivationFunctionType.Exp,
                    accum_out=esum[0:bpc - 1, :],
                )
                nc.vector.reciprocal(rsum[0:bpc - 1, :], esum[0:bpc - 1, :])
                nc.vector.tensor_scalar_mul(
                    a_out[0:bpc - 1, :], p_sb[0:bpc - 1, :], rsum[0:bpc - 1, :])
                nc.sync.dma_start(out=out_a[0:bpc - 1, :], in_=a_out[0:bpc - 1, :])

            pieces = last_split if last else [s // 2, s // 2]
            pstart = 0
            for pi, plen in enumerate(pieces):
                pe_edot = last and pi == len(pieces) - 1
                fsave = []
                g_acc = gpool.tile([128, max_piece], BF, tag="g_acc")
                for m in range(HC):
                    ms = slice(m * 128, (m + 1) * 128)
                    f_m = fpool.tile([128, max_piece], BF, tag=f"f{m}")
                    ps = psf_pool.tile([128, max_piece], FP, tag="ps_f")
                    for gi in range(plen // SLAB):
                        goff = pstart + gi * SLAB
                        gsl = slice(goff, goff + SLAB)
                        n = 0
                        for xa, wb in ((xs["h"], wh_sb["h"]),
                                       (xs["l"], wh_sb["h"]),
                                       (xs["h"], wh_sb["l"])):
                            for cp in range(HC // 2):
                                nc.tensor.matmul(
                                    ps[:, gi * SLAB:(gi + 1) * SLAB],
                                    wb[:, 2 * cp:2 * cp + 2, ms],
                                    xa[:, 2 * cp:2 * cp + 2, gsl],
                                    start=(n == 0),
                                    stop=(n == 5),
                                    perf_mode=DR,
                                )
                                n += 1
                    nc.scalar.activation(
                        out=f_m[:, 0:plen], in_=ps[:, 0:plen],
                        func=mybir.ActivationFunctionType.Tanh,
                        bias=at_sb[m][:, b:b + 1],
                        scale=float(2.0 ** -(KX + KW)),
                    )
                    if pe_edot:
                        fsave.append(f_m)
                    elif m == 0:
                        nc.vector.tensor_scalar_mul(
                            g_acc[:, 0:plen], f_m[:, 0:plen], vwt_sb[:, 0:1])
                    else:
                        # g += f_m * vw_m, fused on DVE
                        nc.vector.scalar_tensor_tensor(
                            g_acc[:, 0:plen], f_m[:, 0:plen],
                            vwt_sb[:, m:m + 1], g_acc[:, 0:plen],
                            op0=mybir.AluOpType.mult,
                            op1=mybir.AluOpType.add,
                        )
                if pe_edot:
                    ps_e = pse_pool.tile([1, SLAB], FP, tag="ps_e")
                    for m in range(HC):
                        nc.tensor.matmul(
                            ps_e[:, 0:plen],
                            vwb_sb[:, m:m + 1],
                            fsave[m][:, 0:plen],
                            start=(m == 0),
                            stop=(m == HC - 1),
                        )
                    nc.scalar.activation(
                        out=p7[pi][:, 0:plen], in_=ps_e[0:1, 0:plen],
                        func=mybir.ActivationFunctionType.Exp,
                        accum_out=s7[pi],
                    )
                    pstart += plen
                    continue
                er = epool.tile([128, max_piece], F16, tag="er")
                nc.gpsimd.partition_all_reduce(
                    er[:, 0:plen], g_acc[:, 0:plen], 128, bass_isa.ReduceOp.add)
                if not last:
                    # all-reduce output is replicated: row-DMA partition 0
                    if "row" not in SKIP:
                        eng = nc.gpsimd if ROWDMA_GPSIMD else nc.sync
                        eng.dma_start(
                            out=e_sb[b:b + 1, pstart:pstart + plen],
                            in_=er[0:1, 0:plen])
                elif "b7sm" in SKIP:
                    pass
                else:
                    nc.scalar.activation(
                        out=p7[pi][:, 0:plen], in_=er[0:1, 0:plen],
                        func=mybir.ActivationFunctionType.Exp,
                        accum_out=s7[pi],
                    )
                pstart += plen

        # batch-7 epilogue: combine pieces, normalize, one output DMA
        if "b7sm" in SKIP:
            nc.sync.dma_start(out=out_a[bpc - 1:bpc, :], in_=a7)
            skip_epilogue = True
        else:
            skip_epilogue = False
        if not skip_epilogue:
            nc.vector.tensor_add(ssum, s7[0], s7[1])
        if not skip_epilogue:
            for i in range(2, n_lp):
                nc.vector.tensor_add(ssum, ssum, s7[i])
            nc.vector.reciprocal(rs7, ssum)
            pstart = 0
            for pi, plen in enumerate(last_split):
                dst = a7[:, pstart:pstart + plen]
                srcp = p7[pi][:, 0:plen]
                # fan the normalizing muls across three engines in parallel
                if pi == 0:
                    nc.vector.tensor_scalar_mul(dst, srcp, rs7)
                elif pi == 1:
                    nc.scalar.mul(dst, srcp, rs7)
                else:
                    nc.gpsimd.tensor_scalar_mul(dst, srcp, rs7)
                pstart += plen
            nc.sync.dma_start(out=out_a[bpc - 1:bpc, :], in_=a7)

    return nc


_PROG_CACHE = {}


def _get_program(key=(BPC, S)):
    if key not in _PROG_CACHE:
        nc = build_program(*key)
        nc.finalize()
        _PROG_CACHE[key] = nc
    return _PROG_CACHE[key]


E4 = ml_dtypes.float8_e4m3


def _q8(v, k):
    """RNE-quantize v*2^k to TRN e4m3 (max +-240); returns float32 array
    still in the scaled domain plus the uint8 bit pattern."""
    s = np.float32(2.0 ** k)
    q = np.clip(v * s, -240.0, 240.0).astype(E4)
    return q.astype(np.float32), q.view(np.uint8)


def make_in_maps(encoder_output, decoder_hidden, coverage, Wh, bh, Ws, bs, Wc, bc,
                 v_w, v_b=None):
    f32 = np.float32
    enc = np.asarray(encoder_output, dtype=f32)
    cov = np.asarray(coverage, dtype=f32)
    Wh64 = np.asarray(Wh, dtype=np.float64)
    # u @ Wh == Wc[0] exactly (f64 solve) -> coverage folds into x
    u = np.linalg.solve(Wh64.T, np.asarray(Wc, dtype=np.float64)[0])
    A = (np.asarray(decoder_hidden, dtype=np.float64)
         @ np.asarray(Ws, dtype=np.float64)
         + np.asarray(bh, dtype=np.float64)
         + np.asarray(bs, dtype=np.float64)
         + np.asarray(bc, dtype=np.float64)).astype(f32)  # [B, H]

    Whf = np.asarray(Wh, dtype=f32)
    whh_f, whh_u8 = _q8(Whf, KW)
    whl_f, whl_u8 = _q8(Whf - whh_f / np.float32(2.0 ** KW), KW)

    def chunked(a2d):
        # [128p, 4c * N] layout with [p, c, n] = a2d[c*128+p, n]
        n = a2d.shape[1]
        return np.ascontiguousarray(
            a2d.reshape(HC, 128, n).transpose(1, 0, 2)).reshape(128, HC * n)

    vw = np.asarray(v_w, dtype=f32).reshape(HC, 128)
    shared = {
        "whh": chunked(whh_u8),
        "whl": chunked(whl_u8),
        "vwt": np.ascontiguousarray(vw.T),
        "vwb": np.ascontiguousarray(vw.T).astype(ml_dtypes.bfloat16).view(np.uint16),
    }
    uf = u.astype(f32)
    in_maps = []
    for c in range(N_CORES):
        lo, hi = c * BPC, (c + 1) * BPC
        xf = enc[lo:hi] + cov[lo:hi][:, :, None] * uf          # [bpc, S, H]
        xt = np.ascontiguousarray(xf.transpose(0, 2, 1))       # [bpc, H, S]
        xh_f, xh_u8 = _q8(xt, KX)
        _, xl_u8 = _q8(xt - xh_f / np.float32(2.0 ** KX), KX)

        def xpack(u8):
            # [bpc*128, 4*S] with row b*128+p holding [c, s] = x[c*128+p, s]
            return np.ascontiguousarray(
                u8.reshape(BPC, HC, 128, S).transpose(0, 2, 1, 3)
            ).reshape(BPC * 128, HC * S)

        m = dict(shared)
        m["xh"] = xpack(xh_u8)
        m["xl"] = xpack(xl_u8)
        m["at"] = np.ascontiguousarray(A[lo:hi].T)             # [H, bpc]
        in_maps.append(m)
    return in_maps


def run_spmd(in_maps, trace=False, **kw):
    from concourse.bass_utils import run_bass_kernel_spmd
    nc = _get_program()
    return run_bass_kernel_spmd(nc, in_maps, core_ids=list(range(N_CORES)),
                                trace=trace, **kw)


def kernel(**inputs) -> tuple[np.ndarray, np.ndarray]:
    in_maps = make_in_maps(**inputs)
    res = run_spmd(in_maps)
    a_t = np.concatenate([r["out_a"] for r in res.results], axis=0)
    a_t = a_t.astype(np.float32)
    cov = np.asarray(inputs["coverage"], dtype=np.float32)
    return a_t, cov + a_t
